# revision 16
# baseline (speedup 1.0000x reference)
"""Trainium2 Bass kernel for the EquivariantGNNBlock problem.

Strategy (data-parallel over molecules, 8 per core x 8 cores):

The expensive part of the reference is the edge MLP: for each edge e,
feat = [h[ii], h[jj], d^2, a] @ We1/Wx1 with a 517-wide contraction.
Because the h-gather is linear, we factor it through the node axis:

    h_i @ We1[0:256] = onehot_i @ (h @ We1[0:256])

so per molecule we precompute Pstack = [[h@W_A], [h@W_B]] (128 x 512,
W_A/W_B are the We1/Wx1 top/bottom halves side by side) and evaluate
layer 1 as ONE K=128 matmul with a stacked one-hot rhs (64 ii rows +
64 jj rows), plus a K=7 matmul for the d^2 and edge-attr rows.  This
replaces a K=517 contraction per edge with a K=135 one.

Layout: features on partitions, edges on the free axis, TE=512 edges
per tile (E=4032 padded to 4096; padded edges get one-hot index -1 so
they match nothing and scatter zero).  The scatter back to nodes is a
matmul against the per-edge-partition one-hot, accumulated in PSUM and
drained to an SBUF accumulator per edge tile.
"""

import numpy as np

import concourse.bass as bass
import concourse.mybir as mybir
import concourse.tile as tile
from concourse.bass_utils import run_bass_kernel_spmd
from concourse.masks import make_identity
from concourse.vector_clock import ScopedClock

F32 = mybir.dt.float32
F32R = mybir.dt.float32r
I32 = mybir.dt.int32
AF = mybir.ActivationFunctionType
OP = mybir.AluOpType

B, N, E, H, A = 64, 64, 4032, 256, 4
NCORES = 8
BM = B // NCORES          # molecules per core
TE = 512                  # edges per tile (free dim)
NEC = 8                   # edge tiles per molecule (last one has 448 real edges)
GRP = 4                   # edge tiles per sqrt-batching group
SCALE = 15.0

# All matmul operands are float32r: the PE's single-pass fp32 mode (4x
# faster than fp32 at free-dim >= 256, ~1.6e-4 measured matmul rel-err,
# i.e. TF32-grade).  The BIR verifier requires f32r matmul operands to be
# PRODUCED as f32r, so those tiles are typed f32r and filled via casting
# (gpsimd) DMAs / ACT / DVE writes.  fp32 fallback: set DT = F32.
DT = F32R


def _install_tilefix():
    """This walrus build allows only one semaphore wait per CTRL instruction;
    Tile's end-of-context drain collects one wait per used logical processor
    onto a single Drain ("Too many sync wait commands").  Split them onto
    individual SP wait instructions instead."""
    if getattr(tile.TileContext, "_drain_split_installed", False):
        return

    def _drain_and_barrier(self, tick_clock, wait_clock):
        carrier = self.nc.sync.nop()
        wait_clock.add_sem_waits(
            carrier.ins, ScopedClock({None: tick_clock.global_clock})
        )
        si = carrier.ins.sync_info
        waits = list(si.on_wait)
        if len(waits) > 1:
            si.on_wait = [waits[0]]
            by_num = {h.num: h for h in self.sems.allocated().values()}
            for w in waits[1:]:
                sem = by_num[w.id]
                self.nc.sync.nop()._wait_ge(sem, w.wait_value)
        self.nc.sync.drain()
        self.nc.all_engine_barrier()
        popped = self.nc._tile_sem_poison_stack.pop()
        assert popped is self._sem_poison
        self.nc.clear_and_free_semaphores(list(self.sems.allocated().values()))
        self.nc.all_engine_barrier()

    tile.TileContext._drain_and_barrier = _drain_and_barrier
    tile.TileContext._drain_split_installed = True

    # Same walrus restriction for every other instruction: split multi-wait
    # instructions at the BIR-JSON level into single-wait NoOps.
    import orjson
    import concourse.bass_utils as bu
    import concourse.bass2jax as bj

    def split_multiwait(bir_json: bytes) -> bytes:
        bir = orjson.loads(bir_json)
        n = 0
        for fn in bir["functions"]:
            for blk in fn["blocks"]:
                out = []
                for inst in blk["instructions"]:
                    si = inst.get("sync_info")
                    waits = (si or {}).get("on_wait") or []
                    if len(waits) > 1:
                        for w in waits[:-1]:
                            n += 1
                            nop = {
                                "engine": inst["engine"], "ins": [], "outs": [],
                                "name": f"WSPLIT-{n}", "opcode": "NoOp",
                                "sync_info": {"on_update": [], "on_wait": [w]},
                            }
                            if "debug" in inst:
                                nop["debug"] = inst["debug"]
                            out.append(nop)
                        si["on_wait"] = [waits[-1]]
                    out.append(inst)
                blk["instructions"] = out
        return orjson.dumps(bir)

    orig = bu.compile_bir_kernel

    def patched_compile(bir_json, tmpdir, neff_name="file.neff"):
        return orig(split_multiwait(bir_json), tmpdir, neff_name)

    bu.compile_bir_kernel = patched_compile
    bj.compile_bir_kernel = patched_compile


def build_nc(repeat: int = 1) -> bass.Bass:
    _install_tilefix()
    nc = bass.Bass()

    x_in = nc.declare_dram_parameter("x_in", [BM, N, 3], F32, isOutput=False)
    h_in = nc.declare_dram_parameter("h_in", [BM, N, H], F32, isOutput=False)
    a_in = nc.declare_dram_parameter("a_in", [BM, E, A], F32, isOutput=False)
    ei_in = nc.declare_dram_parameter("ei_in", [BM, E, 2], I32, isOutput=False)
    nm_in = nc.declare_dram_parameter("nm_in", [BM, N], F32, isOutput=False)
    em_in = nc.declare_dram_parameter("em_in", [BM, E], F32, isOutput=False)
    W_A_in = nc.declare_dram_parameter("W_A", [H, 512], F32, isOutput=False)
    W_B_in = nc.declare_dram_parameter("W_B", [H, 512], F32, isOutput=False)
    W_sqa_in = nc.declare_dram_parameter("W_sqa", [7, 512], F32, isOutput=False)
    We2_in = nc.declare_dram_parameter("We2", [H, H], F32, isOutput=False)
    Wx2_in = nc.declare_dram_parameter("Wx2", [H, H], F32, isOutput=False)
    Wh1a_in = nc.declare_dram_parameter("Wh1a", [H, H], F32, isOutput=False)
    Wh1b_in = nc.declare_dram_parameter("Wh1b", [H, H], F32, isOutput=False)
    Wh2_in = nc.declare_dram_parameter("Wh2", [H, H], F32, isOutput=False)
    Wa_r_in = nc.declare_dram_parameter("Wa_rep", [H, 128], F32, isOutput=False)
    Wx3_r_in = nc.declare_dram_parameter("Wx3_rep", [H, 128], F32, isOutput=False)
    bias_in = nc.declare_dram_parameter("BIAS", [128, 13], F32, isOutput=False)
    ones_in = nc.declare_dram_parameter("ONES", [3, 128], F32, isOutput=False)
    sel2_in = nc.declare_dram_parameter("SEL2", [2, 128], F32, isOutput=False)

    x_out = nc.declare_dram_parameter("x_out", [BM, N, 3], F32, isOutput=True)
    h_out = nc.declare_dram_parameter("h_out", [BM, N, H], F32, isOutput=True)

    # bias column indices in the BIAS matrix
    BE1, BX1, BE2, BX2, BH1, BH2, BA = 0, 2, 4, 6, 8, 10, 12

    with tile.TileContext(nc) as tc:
        with (
            tc.tile_pool(name="consts", bufs=1) as cpool,
            tc.tile_pool(name="mol", bufs=2) as mpool,
            tc.tile_pool(name="edge", bufs=2) as epool,
            tc.tile_pool(name="eper", bufs=GRP + 1) as ppool,
            tc.tile_pool(name="pbig", bufs=4, space="PSUM") as pbig,
            tc.tile_pool(name="pgeo", bufs=1, space="PSUM") as pgeo,
            tc.tile_pool(name="pt", bufs=2, space="PSUM") as ppt,
            tc.tile_pool(name="pagg", bufs=1, space="PSUM") as pagg,
        ):
            # ---- constants ----
            ident = cpool.tile([128, 128], F32)
            make_identity(nc, ident[:])
            ones3 = cpool.tile([3, 3], DT)
            nc.gpsimd.dma_start(out=ones3[:], in_=ones_in[0:3, 0:3])
            sel2 = cpool.tile([2, 128], DT)
            nc.gpsimd.dma_start(out=sel2[:], in_=sel2_in[:])
            iota2_i = cpool.tile([128, 1], I32)
            nc.gpsimd.iota(iota2_i[:], pattern=[[0, 1]], base=0, channel_multiplier=1)
            iota2 = cpool.tile([128, 1], F32)
            nc.vector.tensor_copy(out=iota2[:], in_=iota2_i[:])
            nc.vector.tensor_scalar(
                out=iota2[64:128, :], in0=iota2[64:128, :], scalar1=-64.0,
                scalar2=None, op0=OP.add,
            )
            iotam_i = cpool.tile([128, 64], I32)
            nc.gpsimd.iota(iotam_i[:], pattern=[[1, 64]], base=0, channel_multiplier=0)
            iotam = cpool.tile([128, 64], F32)
            nc.vector.tensor_copy(out=iotam[:], in_=iotam_i[:])

            BIAS = cpool.tile([128, 13], F32)
            nc.sync.dma_start(out=BIAS[:], in_=bias_in[:])

            def load_w(dram, rows, cols):
                tiles = []
                for kc in range(rows // 128):
                    t = cpool.tile([128, cols], DT, tag=f"w_{dram.name}_{kc}")
                    nc.gpsimd.dma_start(out=t[:], in_=dram[kc * 128:(kc + 1) * 128, :])
                    tiles.append(t)
                return tiles

            W_A = load_w(W_A_in, H, 512)
            W_B = load_w(W_B_in, H, 512)
            We2 = load_w(We2_in, H, H)
            Wx2 = load_w(Wx2_in, H, H)
            Wh1a = load_w(Wh1a_in, H, H)
            Wh1b = load_w(Wh1b_in, H, H)
            Wh2 = load_w(Wh2_in, H, H)
            Wa_r = load_w(Wa_r_in, H, 128)
            Wx3_r = load_w(Wx3_r_in, H, 128)
            W_sqa = cpool.tile([7, 512], DT)
            nc.gpsimd.dma_start(out=W_sqa[:], in_=W_sqa_in[:])

            for m in [mm for _ in range(repeat) for mm in range(BM)]:
                # ---- per-molecule node-side setup ----
                xs = mpool.tile([128, 3], DT, tag="xs")
                nc.gpsimd.dma_start(out=xs[0:64, :], in_=x_in[m])
                nc.gpsimd.dma_start(out=xs[64:128, :], in_=x_in[m])
                nc.vector.tensor_scalar(
                    out=xs[64:128, :], in0=xs[64:128, :], scalar1=-1.0,
                    scalar2=None, op0=OP.mult,
                )

                h_sb = mpool.tile([64, H], F32, tag="h_sb")
                nc.sync.dma_start(out=h_sb[:], in_=h_in[m])
                hT = []
                for kc in range(2):
                    pt = ppt.tile([128, 128], F32, tag="pt")
                    nc.tensor.transpose(
                        out=pt[:, 0:64],
                        in_=h_sb[:, kc * 128:(kc + 1) * 128],
                        identity=ident[0:64, 0:64],
                    )
                    t = mpool.tile([128, 64], DT, tag=f"hT{kc}")
                    nc.vector.tensor_copy(out=t[:], in_=pt[:, 0:64])
                    hT.append(t)

                # Pstack = [h @ W_A ; h @ W_B]  (lhsT for the L1 one-hot matmul).
                # f32r matmuls must write psum at partition 0, so the W_B half
                # bounces through SBUF and a partition-shifting DMA.
                Pstack = mpool.tile([128, 512], DT, tag="Pstack")
                ptmp = mpool.tile([64, 512], DT, tag="ptmp")
                for half, W in ((0, W_A), (1, W_B)):
                    pp = pbig.tile([128, 512], F32, tag="pbig")
                    for kc in range(2):
                        nc.tensor.matmul(
                            out=pp[0:64, :], lhsT=hT[kc][:], rhs=W[kc][:],
                            start=(kc == 0), stop=(kc == 1),
                        )
                    if half == 0:
                        nc.vector.tensor_copy(out=Pstack[0:64, :], in_=pp[0:64, :])
                    else:
                        nc.vector.tensor_copy(out=ptmp[:], in_=pp[0:64, :])
                        nc.sync.dma_start(out=Pstack[64:128, :], in_=ptmp[:])

                aggc = mpool.tile([64, 260], F32, tag="aggc")
                nc.gpsimd.memset(aggc[:], 0.0)

                for g in range(NEC // GRP):
                    stash = []
                    for ec in range(g * GRP, (g + 1) * GRP):
                        e0 = ec * TE
                        ne = min(TE, E - e0)
                        last = ne < TE

                        # ---- indices ----
                        iijj_r = epool.tile([2, TE], I32, tag="iijj_r")
                        if last:
                            nc.gpsimd.memset(iijj_r[:, ne:TE], -1)
                        nc.sync.dma_start(
                            out=iijj_r[0:1, 0:ne], in_=ei_in[m, e0:e0 + ne, 0][None, :]
                        )
                        nc.sync.dma_start(
                            out=iijj_r[1:2, 0:ne], in_=ei_in[m, e0:e0 + ne, 1][None, :]
                        )
                        iijj_f = epool.tile([2, TE], DT, tag="iijj_f")
                        nc.vector.tensor_copy(out=iijj_f[:], in_=iijj_r[:])

                        iic_i = epool.tile([128, 4], I32, tag="iic_i")
                        if last:
                            nc.gpsimd.memset(iic_i[:], -1)
                            nc.sync.dma_start(
                                out=iic_i[:, 0:3],
                                in_=ei_in[m, e0:e0 + 384, 0].rearrange(
                                    "(c p) -> p c", p=128
                                ),
                            )
                            nc.sync.dma_start(
                                out=iic_i[0:64, 3:4],
                                in_=ei_in[m, e0 + 384:e0 + 448, 0][:, None],
                            )
                        else:
                            nc.sync.dma_start(
                                out=iic_i[:],
                                in_=ei_in[m, e0:e0 + TE, 0].rearrange(
                                    "(c p) -> p c", p=128
                                ),
                            )
                        iic = epool.tile([128, 4], F32, tag="iic")
                        nc.vector.tensor_copy(out=iic[:], in_=iic_i[:])

                        # ---- one-hot (gather layout): rows 0-63 ii, 64-127 jj
                        pb = pbig.tile([128, TE], F32, tag="pbig")
                        nc.tensor.matmul(
                            out=pb[:], lhsT=sel2[:], rhs=iijj_f[:],
                            start=True, stop=True,
                        )
                        OT = epool.tile([128, TE], DT, tag="OT")
                        nc.vector.tensor_scalar(
                            out=OT[:], in0=pb[:], scalar1=iota2[:, 0:1],
                            scalar2=None, op0=OP.is_equal,
                        )

                        # scatter one-hot (edge-partition layout)
                        O_i = ppool.tile([128, 256], DT, tag="O_i")
                        for c in range(4):
                            nc.vector.tensor_scalar(
                                out=O_i[:, c * 64:(c + 1) * 64], in0=iotam[:],
                                scalar1=iic[:, c:c + 1], scalar2=None,
                                op0=OP.is_equal,
                            )

                        # ---- geometry ----
                        mask3 = epool.tile([3, TE], F32, tag="mask3")
                        if last:
                            nc.gpsimd.memset(mask3[:], 0.0)
                        for r in range(3):
                            nc.sync.dma_start(
                                out=mask3[r:r + 1, 0:ne],
                                in_=em_in[m, e0:e0 + ne][None, :],
                            )

                        geo = pgeo.tile([3, TE], F32, tag="geo")
                        nc.tensor.matmul(
                            out=geo[:], lhsT=xs[:], rhs=OT[:],
                            start=True, stop=True,
                        )
                        diff_m = ppool.tile([3, TE], F32, tag="diff_m")
                        nc.vector.tensor_mul(out=diff_m[:], in0=geo[:], in1=mask3[:])

                        sqa = epool.tile([7, TE], DT, tag="sqa")
                        nc.vector.tensor_mul(
                            out=sqa[0:3, :], in0=diff_m[:], in1=diff_m[:]
                        )
                        nc.gpsimd.dma_start(
                            out=sqa[3:7, 0:ne],
                            in_=a_in[m, e0:e0 + ne, :].rearrange("e k -> k e"),
                        )

                        # d^2 into a fresh geo bank, then stash in SBUF for the
                        # batched sqrt phase
                        geo2 = pgeo.tile([3, TE], F32, tag="geo")
                        nc.tensor.matmul(
                            out=geo2[:], lhsT=ones3[:], rhs=sqa[0:3, :],
                            start=True, stop=True,
                        )
                        d2_sb = ppool.tile([3, TE], F32, tag="d2_sb")
                        nc.vector.tensor_copy(out=d2_sb[:], in_=geo2[:])

                        # ---- layer 1 ----
                        m1 = []
                        for mc in range(4):
                            ms = slice(mc * 128, (mc + 1) * 128)
                            pl1 = pbig.tile([128, TE], F32, tag="pbig")
                            nc.tensor.matmul(
                                out=pl1[:], lhsT=Pstack[:, ms], rhs=OT[:],
                                start=True, stop=False,
                            )
                            nc.tensor.matmul(
                                out=pl1[:], lhsT=W_sqa[:, ms], rhs=sqa[:],
                                start=False, stop=True,
                            )
                            t = epool.tile([128, TE], DT, tag=f"m1_{mc}")
                            bcol = (BE1 if mc < 2 else BX1) + (mc % 2)
                            nc.scalar.activation(
                                out=t[:], in_=pl1[:], func=AF.Silu,
                                bias=BIAS[:, bcol:bcol + 1],
                            )
                            m1.append(t)

                        # ---- layer 2 ----
                        m2 = []
                        for path, (W, boff, srcs) in enumerate(
                            ((We2, BE2, m1[0:2]), (Wx2, BX2, m1[2:4]))
                        ):
                            for mc in range(2):
                                ms = slice(mc * 128, (mc + 1) * 128)
                                pl2 = pbig.tile([128, TE], F32, tag="pbig")
                                for kc in range(2):
                                    nc.tensor.matmul(
                                        out=pl2[:], lhsT=W[kc][:, ms],
                                        rhs=srcs[kc][:],
                                        start=(kc == 0), stop=(kc == 1),
                                    )
                                t = epool.tile([128, TE], DT, tag=f"m2_{path}_{mc}")
                                nc.scalar.activation(
                                    out=t[:], in_=pl2[:], func=AF.Silu,
                                    bias=BIAS[:, boff + mc:boff + mc + 1],
                                )
                                m2.append(t)
                        m2e, m2x = m2[0:2], m2[2:4]

                        # ---- gates (sigmoid via tanh: sigmoid(z) =
                        # 0.5*tanh(z/2) + 0.5 keeps the silu/tanh ACT table
                        # resident; BIAS col BA holds ba/2) ----
                        pg = pbig.tile([128, TE], F32, tag="pbig")
                        for kc in range(2):
                            nc.tensor.matmul(
                                out=pg[:], lhsT=Wa_r[kc][:], rhs=m2e[kc][:],
                                start=(kc == 0), stop=(kc == 1),
                            )
                        gate = epool.tile([128, TE], F32, tag="gate")
                        nc.scalar.activation(
                            out=gate[:], in_=pg[:], func=AF.Tanh,
                            bias=BIAS[:, BA:BA + 1], scale=0.5,
                        )
                        nc.vector.tensor_scalar(
                            out=gate[:], in0=gate[:], scalar1=0.5, scalar2=0.5,
                            op0=OP.mult, op1=OP.add,
                        )
                        em = []
                        for kc in range(2):
                            t = ppool.tile([128, TE], F32, tag=f"em_{kc}")
                            nc.vector.tensor_mul(out=t[:], in0=m2e[kc][:], in1=gate[:])
                            em.append(t)

                        px = pbig.tile([128, TE], F32, tag="pbig")
                        for kc in range(2):
                            nc.tensor.matmul(
                                out=px[:], lhsT=Wx3_r[kc][:], rhs=m2x[kc][:],
                                start=(kc == 0), stop=(kc == 1),
                            )
                        xw3 = ppool.tile([3, TE], F32, tag="xw3")
                        nc.scalar.activation(out=xw3[:], in_=px[0:3, :], func=AF.Tanh)

                        stash.append((em, O_i, diff_m, d2_sb, xw3))

                    # ---- phase B: batched sqrt + xm + scatter for the group ----
                    for gi, (em, O_i, diff_m, d2_sb, xw3) in enumerate(stash):
                        r15 = epool.tile([3, TE], F32, tag="r15")
                        nc.scalar.sqrt(out=r15[:], in_=d2_sb[:])
                        nc.vector.tensor_scalar(
                            out=r15[:], in0=r15[:], scalar1=1.0, scalar2=None,
                            op0=OP.add,
                        )
                        nc.vector.reciprocal(out=r15[:], in_=r15[:])

                        xm = epool.tile([4, TE], F32, tag="xm")
                        nc.gpsimd.memset(xm[:], 0.0)
                        nc.vector.tensor_mul(out=xm[0:3, :], in0=xw3[:], in1=diff_m[:])
                        nc.vector.tensor_mul(out=xm[0:3, :], in0=xm[0:3, :], in1=r15[:])
                        nc.vector.tensor_scalar(
                            out=xm[0:3, :], in0=xm[0:3, :], scalar1=SCALE,
                            scalar2=None, op0=OP.mult,
                        )

                        # em scatter group (cols 0:256), completed before the
                        # xm group starts: start=True resets has_written for the
                        # whole bank, so groups in one bank must be sequential.
                        psc = pagg.tile([64, 260], F32, tag="pagg")
                        xmxs = []
                        for c in range(4):
                            cs = slice(c * 128, (c + 1) * 128)
                            emx = epool.tile([128, 256], DT, tag="emx")
                            for kc in range(2):
                                pt = ppt.tile([128, 128], F32, tag="pt")
                                nc.tensor.transpose(
                                    out=pt[:], in_=em[kc][:, cs], identity=ident[:],
                                )
                                nc.vector.tensor_copy(
                                    out=emx[:, kc * 128:(kc + 1) * 128], in_=pt[:]
                                )
                            nc.tensor.matmul(
                                out=psc[:, 0:256], lhsT=O_i[:, c * 64:(c + 1) * 64],
                                rhs=emx[:], start=(c == 0), stop=(c == 3),
                                skip_group_check=True,
                            )
                            ptx = ppt.tile([128, 128], F32, tag="pt")
                            nc.tensor.transpose(
                                out=ptx[:, 0:4], in_=xm[:, cs],
                                identity=ident[0:4, 0:4],
                            )
                            xmx = epool.tile([128, 4], DT, tag=f"xmx{c}")
                            nc.vector.tensor_copy(out=xmx[:], in_=ptx[:, 0:4])
                            xmxs.append(xmx)
                        for c in range(4):
                            nc.tensor.matmul(
                                out=psc[:, 256:260], lhsT=O_i[:, c * 64:(c + 1) * 64],
                                rhs=xmxs[c][:], start=(c == 0), stop=(c == 3),
                                skip_group_check=True,
                            )
                        nc.vector.tensor_add(out=aggc[:], in0=aggc[:], in1=psc[:])

                # ---- node MLP + outputs ----
                em_aggT = []
                for kc in range(2):
                    pt = ppt.tile([128, 128], F32, tag="pt")
                    nc.tensor.transpose(
                        out=pt[:, 0:64],
                        in_=aggc[:, kc * 128:(kc + 1) * 128],
                        identity=ident[0:64, 0:64],
                    )
                    t = mpool.tile([128, 64], DT, tag=f"em_aggT{kc}")
                    nc.vector.tensor_copy(out=t[:], in_=pt[:, 0:64])
                    em_aggT.append(t)

                t1s = []
                for mc in range(2):
                    ms = slice(mc * 128, (mc + 1) * 128)
                    pz = ppt.tile([128, 128], F32, tag="pt")
                    for kc in range(2):
                        nc.tensor.matmul(
                            out=pz[:, 0:64], lhsT=Wh1a[kc][:, ms], rhs=hT[kc][:],
                            start=(kc == 0), stop=False,
                        )
                    for kc in range(2):
                        nc.tensor.matmul(
                            out=pz[:, 0:64], lhsT=Wh1b[kc][:, ms],
                            rhs=em_aggT[kc][:],
                            start=False, stop=(kc == 1),
                        )
                    t = mpool.tile([128, 64], DT, tag=f"t1s{mc}")
                    nc.scalar.activation(
                        out=t[:], in_=pz[:, 0:64], func=AF.Silu,
                        bias=BIAS[:, BH1 + mc:BH1 + mc + 1],
                    )
                    t1s.append(t)

                t2 = mpool.tile([64, H], F32, tag="t2")
                for mc in range(2):
                    ms = slice(mc * 128, (mc + 1) * 128)
                    pz = ppt.tile([128, 128], F32, tag="pt")
                    for kc in range(2):
                        nc.tensor.matmul(
                            out=pz[:, 0:64], lhsT=Wh2[kc][:, ms], rhs=t1s[kc][:],
                            start=(kc == 0), stop=(kc == 1),
                        )
                    t2T = mpool.tile([128, 64], F32, tag=f"t2T{mc}")
                    nc.vector.tensor_scalar(
                        out=t2T[:], in0=pz[:, 0:64],
                        scalar1=BIAS[:, BH2 + mc:BH2 + mc + 1],
                        scalar2=None, op0=OP.add,
                    )
                    pb2 = ppt.tile([128, 128], F32, tag="pt")
                    nc.tensor.transpose(
                        out=pb2[0:64, :], in_=t2T[:], identity=ident[:],
                    )
                    nc.vector.tensor_copy(out=t2[:, ms], in_=pb2[0:64, :])

                nm = mpool.tile([64, 1], F32, tag="nm")
                nc.sync.dma_start(out=nm[:], in_=nm_in[m, :][:, None])

                ho = mpool.tile([64, H], F32, tag="ho")
                nc.vector.tensor_add(out=ho[:], in0=h_sb[:], in1=t2[:])
                nc.vector.tensor_scalar(
                    out=ho[:], in0=ho[:], scalar1=nm[:, 0:1], scalar2=None,
                    op0=OP.mult,
                )
                nc.sync.dma_start(out=h_out[m], in_=ho[:])

                xo = mpool.tile([64, 3], F32, tag="xo")
                nc.vector.tensor_add(out=xo[:], in0=xs[0:64, :], in1=aggc[:, 256:259])
                nc.vector.tensor_scalar(
                    out=xo[:], in0=xo[:], scalar1=nm[:, 0:1], scalar2=None,
                    op0=OP.mult,
                )
                nc.sync.dma_start(out=x_out[m], in_=xo[:])

    return nc


_NC_CACHE = None


def _get_nc():
    global _NC_CACHE
    if _NC_CACHE is None:
        _NC_CACHE = build_nc()
    return _NC_CACHE


def make_in_maps(x, h, a, edge_indices, node_mask, edge_mask,
                 We1, be1, We2, be2, Wa, ba,
                 Wh1, bh1, Wh2, bh2, Wx1, bx1, Wx2, bx2, Wx3):
    f32 = np.float32
    W_A = np.concatenate([We1[0:256], Wx1[0:256]], axis=1).astype(f32)
    W_B = np.concatenate([We1[256:512], Wx1[256:512]], axis=1).astype(f32)
    W_sqa = np.zeros((7, 512), f32)
    W_sqa[0:3, :] = np.concatenate([We1[512:513], Wx1[512:513]], axis=1)
    W_sqa[3:7, :] = np.concatenate([We1[513:517], Wx1[513:517]], axis=1)
    Wa_rep = np.repeat(Wa.astype(f32), 128, axis=1)
    Wx3_rep = np.repeat(Wx3.astype(f32), 128, axis=1)
    BIAS = np.zeros((128, 13), f32)
    for col, vec in ((0, be1), (2, bx1), (4, be2), (6, bx2), (8, bh1), (10, bh2)):
        BIAS[:, col] = vec[0:128]
        BIAS[:, col + 1] = vec[128:256]
    BIAS[:, 12] = 0.5 * ba[0]

    ONES = np.ones((3, 128), f32)
    SEL2 = np.zeros((2, 128), f32)
    SEL2[0, 0:64] = 1.0   # broadcast row 0 (ii) to partitions 0-63
    SEL2[1, 64:128] = 1.0  # broadcast row 1 (jj) to partitions 64-127
    shared = dict(
        W_A=W_A, W_B=W_B, W_sqa=W_sqa,
        We2=We2.astype(f32), Wx2=Wx2.astype(f32),
        Wh1a=Wh1[0:256].astype(f32), Wh1b=Wh1[256:512].astype(f32),
        Wh2=Wh2.astype(f32), Wa_rep=Wa_rep, Wx3_rep=Wx3_rep, BIAS=BIAS,
        ONES=ONES, SEL2=SEL2,
    )
    in_maps = []
    for c in range(NCORES):
        s = slice(c * BM, (c + 1) * BM)
        in_maps.append(dict(
            x_in=np.ascontiguousarray(x[s], f32),
            h_in=np.ascontiguousarray(h[s], f32),
            a_in=np.ascontiguousarray(a[s], f32),
            ei_in=np.ascontiguousarray(edge_indices[s], np.int32),
            nm_in=np.ascontiguousarray(node_mask[s, :, 0], f32),
            em_in=np.ascontiguousarray(edge_mask[s, :, 0], f32),
            **shared,
        ))
    return in_maps


def kernel(**inputs):
    inputs = {k: np.asarray(v) for k, v in inputs.items()}
    nc = _get_nc()
    in_maps = make_in_maps(**inputs)
    res = run_bass_kernel_spmd(nc, in_maps, list(range(NCORES)))
    x_out = np.concatenate([res.results[c]["x_out"] for c in range(NCORES)], axis=0)
    h_out = np.concatenate([res.results[c]["h_out"] for c in range(NCORES)], axis=0)
    return x_out.astype(np.float32), h_out.astype(np.float32)


# revision 17
# speedup vs baseline: 3.1581x; 3.1581x over previous
"""Trainium2 Bass kernel for the EquivariantGNNBlock problem.

Strategy (data-parallel over molecules, 8 per core x 8 cores):

The expensive part of the reference is the edge MLP: for each edge e,
feat = [h[ii], h[jj], d^2, a] @ We1/Wx1 with a 517-wide contraction.
Because the h-gather is linear, we factor it through the node axis:

    h_i @ We1[0:256] = onehot_i @ (h @ We1[0:256])

so per molecule we precompute Pstack = [[h@W_A], [h@W_B]] (128 x 512,
W_A/W_B are the We1/Wx1 top/bottom halves side by side) and evaluate
layer 1 as ONE K=128 matmul with a stacked one-hot rhs (64 ii rows +
64 jj rows), plus a K=7 matmul for the d^2 and edge-attr rows.  This
replaces a K=517 contraction per edge with a K=135 one.

Layout: features on partitions, edges on the free axis, TE=512 edges
per tile (E=4032 padded to 4096; padded edges get one-hot index -1 so
they match nothing and scatter zero).  The scatter back to nodes is a
matmul against the per-edge-partition one-hot, accumulated in PSUM and
drained to an SBUF accumulator per edge tile.
"""

import numpy as np

import concourse.bass as bass
import concourse.mybir as mybir
import concourse.tile as tile
from concourse.bass_utils import run_bass_kernel_spmd
from concourse.masks import make_identity
from concourse.vector_clock import ScopedClock

F32 = mybir.dt.float32
F32R = mybir.dt.float32r
I32 = mybir.dt.int32
AF = mybir.ActivationFunctionType
OP = mybir.AluOpType

B, N, E, H, A = 64, 64, 4032, 256, 4
NCORES = 8
BM = B // NCORES          # molecules per core
TE = 512                  # edges per tile (free dim)
NEC = 8                   # edge tiles per molecule (last one has 448 real edges)
GRP = 4                   # edge tiles per sqrt-batching group
SCALE = 15.0

# All matmul operands are float32r: the PE's single-pass fp32 mode (4x
# faster than fp32 at free-dim >= 256, ~1.6e-4 measured matmul rel-err,
# i.e. TF32-grade).  The BIR verifier requires f32r matmul operands to be
# PRODUCED as f32r, so those tiles are typed f32r and filled via casting
# (gpsimd) DMAs / ACT / DVE writes.  fp32 fallback: set DT = F32.
DT = F32R


def _install_tilefix():
    """This walrus build allows only one semaphore wait per CTRL instruction;
    Tile's end-of-context drain collects one wait per used logical processor
    onto a single Drain ("Too many sync wait commands").  Split them onto
    individual SP wait instructions instead."""
    if getattr(tile.TileContext, "_drain_split_installed", False):
        return

    def _drain_and_barrier(self, tick_clock, wait_clock):
        carrier = self.nc.sync.nop()
        wait_clock.add_sem_waits(
            carrier.ins, ScopedClock({None: tick_clock.global_clock})
        )
        si = carrier.ins.sync_info
        waits = list(si.on_wait)
        if len(waits) > 1:
            si.on_wait = [waits[0]]
            by_num = {h.num: h for h in self.sems.allocated().values()}
            for w in waits[1:]:
                sem = by_num[w.id]
                self.nc.sync.nop()._wait_ge(sem, w.wait_value)
        self.nc.sync.drain()
        self.nc.all_engine_barrier()
        popped = self.nc._tile_sem_poison_stack.pop()
        assert popped is self._sem_poison
        self.nc.clear_and_free_semaphores(list(self.sems.allocated().values()))
        self.nc.all_engine_barrier()

    tile.TileContext._drain_and_barrier = _drain_and_barrier
    tile.TileContext._drain_split_installed = True

    # Same walrus restriction for every other instruction: split multi-wait
    # instructions at the BIR-JSON level into single-wait NoOps.
    import orjson
    import concourse.bass_utils as bu
    import concourse.bass2jax as bj

    def split_multiwait(bir_json: bytes) -> bytes:
        bir = orjson.loads(bir_json)
        n = 0
        for fn in bir["functions"]:
            for blk in fn["blocks"]:
                out = []
                for inst in blk["instructions"]:
                    si = inst.get("sync_info")
                    waits = (si or {}).get("on_wait") or []
                    if len(waits) > 1:
                        for w in waits[:-1]:
                            n += 1
                            nop = {
                                "engine": inst["engine"], "ins": [], "outs": [],
                                "name": f"WSPLIT-{n}", "opcode": "NoOp",
                                "sync_info": {"on_update": [], "on_wait": [w]},
                            }
                            if "debug" in inst:
                                nop["debug"] = inst["debug"]
                            out.append(nop)
                        si["on_wait"] = [waits[-1]]
                    out.append(inst)
                blk["instructions"] = out
        return orjson.dumps(bir)

    orig = bu.compile_bir_kernel

    def patched_compile(bir_json, tmpdir, neff_name="file.neff"):
        return orig(split_multiwait(bir_json), tmpdir, neff_name)

    bu.compile_bir_kernel = patched_compile
    bj.compile_bir_kernel = patched_compile


def build_nc(repeat: int = 1) -> bass.Bass:
    _install_tilefix()
    nc = bass.Bass()

    x_in = nc.declare_dram_parameter("x_in", [BM, N, 3], F32, isOutput=False)
    h_in = nc.declare_dram_parameter("h_in", [BM, N, H], F32, isOutput=False)
    a_in = nc.declare_dram_parameter("a_in", [BM, E, A], F32, isOutput=False)
    ei_in = nc.declare_dram_parameter("ei_in", [BM, E, 2], I32, isOutput=False)
    nm_in = nc.declare_dram_parameter("nm_in", [BM, N], F32, isOutput=False)
    em_in = nc.declare_dram_parameter("em_in", [BM, E], F32, isOutput=False)
    W_A_in = nc.declare_dram_parameter("W_A", [H, 512], F32, isOutput=False)
    W_B_in = nc.declare_dram_parameter("W_B", [H, 512], F32, isOutput=False)
    W_sqa_in = nc.declare_dram_parameter("W_sqa", [7, 512], F32, isOutput=False)
    We2_in = nc.declare_dram_parameter("We2", [H, H], F32, isOutput=False)
    Wx2_in = nc.declare_dram_parameter("Wx2", [H, H], F32, isOutput=False)
    Wh1a_in = nc.declare_dram_parameter("Wh1a", [H, H], F32, isOutput=False)
    Wh1b_in = nc.declare_dram_parameter("Wh1b", [H, H], F32, isOutput=False)
    Wh2_in = nc.declare_dram_parameter("Wh2", [H, H], F32, isOutput=False)
    Wa_r_in = nc.declare_dram_parameter("Wa_rep", [H, 128], F32, isOutput=False)
    Wx3_r_in = nc.declare_dram_parameter("Wx3_rep", [H, 128], F32, isOutput=False)
    bias_in = nc.declare_dram_parameter("BIAS", [128, 13], F32, isOutput=False)
    ones_in = nc.declare_dram_parameter("ONES", [3, 128], F32, isOutput=False)
    sel2_in = nc.declare_dram_parameter("SEL2", [2, 128], F32, isOutput=False)

    x_out = nc.declare_dram_parameter("x_out", [BM, N, 3], F32, isOutput=True)
    h_out = nc.declare_dram_parameter("h_out", [BM, N, H], F32, isOutput=True)

    # bias column indices in the BIAS matrix
    BE1, BX1, BE2, BX2, BH1, BH2, BA = 0, 2, 4, 6, 8, 10, 12

    with tile.TileContext(nc) as tc:
        with (
            tc.tile_pool(name="consts", bufs=1) as cpool,
            tc.tile_pool(name="mol", bufs=2) as mpool,
            tc.tile_pool(name="edge", bufs=2) as epool,
            tc.tile_pool(name="eper", bufs=GRP + 1) as ppool,
            tc.tile_pool(name="pbig", bufs=4, space="PSUM") as pbig,
            tc.tile_pool(name="pgeo", bufs=1, space="PSUM") as pgeo,
            tc.tile_pool(name="pt", bufs=2, space="PSUM") as ppt,
            tc.tile_pool(name="pagg", bufs=1, space="PSUM") as pagg,
        ):
            # ---- constants ----
            ident = cpool.tile([128, 128], F32)
            make_identity(nc, ident[:])
            ident_r = cpool.tile([128, 128], DT)
            ones3 = cpool.tile([3, 3], DT)
            nc.gpsimd.dma_start(out=ones3[:], in_=ones_in[0:3, 0:3])
            sel2 = cpool.tile([2, 128], DT)
            nc.gpsimd.dma_start(out=sel2[:], in_=sel2_in[:])
            nc.gpsimd.dma_start(out=ident_r[:], in_=ident[:])
            iota2_i = cpool.tile([128, 1], I32)
            nc.gpsimd.iota(iota2_i[:], pattern=[[0, 1]], base=0, channel_multiplier=1)
            iota2 = cpool.tile([128, 1], F32)
            nc.vector.tensor_copy(out=iota2[:], in_=iota2_i[:])
            nc.vector.tensor_scalar(
                out=iota2[64:128, :], in0=iota2[64:128, :], scalar1=-64.0,
                scalar2=None, op0=OP.add,
            )
            iotam_i = cpool.tile([128, 64], I32)
            nc.gpsimd.iota(iotam_i[:], pattern=[[1, 64]], base=0, channel_multiplier=0)
            iotam = cpool.tile([128, 64], F32)
            nc.vector.tensor_copy(out=iotam[:], in_=iotam_i[:])

            BIAS = cpool.tile([128, 13], F32)
            nc.sync.dma_start(out=BIAS[:], in_=bias_in[:])

            def load_w(dram, rows, cols):
                tiles = []
                for kc in range(rows // 128):
                    t = cpool.tile([128, cols], DT, tag=f"w_{dram.name}_{kc}")
                    nc.gpsimd.dma_start(out=t[:], in_=dram[kc * 128:(kc + 1) * 128, :])
                    tiles.append(t)
                return tiles

            W_A = load_w(W_A_in, H, 512)
            W_B = load_w(W_B_in, H, 512)
            We2 = load_w(We2_in, H, H)
            Wx2 = load_w(Wx2_in, H, H)
            Wh1a = load_w(Wh1a_in, H, H)
            Wh1b = load_w(Wh1b_in, H, H)
            Wh2 = load_w(Wh2_in, H, H)
            Wa_r = load_w(Wa_r_in, H, 128)
            Wx3_r = load_w(Wx3_r_in, H, 128)
            W_sqa = cpool.tile([7, 512], DT)
            nc.gpsimd.dma_start(out=W_sqa[:], in_=W_sqa_in[:])

            for m in [mm for _ in range(repeat) for mm in range(BM)]:
                # ---- per-molecule node-side setup ----
                xs = mpool.tile([128, 3], DT, tag="xs")
                nc.gpsimd.dma_start(out=xs[0:64, :], in_=x_in[m])
                nc.gpsimd.dma_start(out=xs[64:128, :], in_=x_in[m])
                nc.vector.tensor_scalar(
                    out=xs[64:128, :], in0=xs[64:128, :], scalar1=-1.0,
                    scalar2=None, op0=OP.mult,
                )

                h_sb = mpool.tile([64, H], F32, tag="h_sb")
                nc.sync.dma_start(out=h_sb[:], in_=h_in[m])
                hT = []
                for kc in range(2):
                    pt = ppt.tile([128, 128], F32, tag="pt")
                    nc.tensor.transpose(
                        out=pt[:, 0:64],
                        in_=h_sb[:, kc * 128:(kc + 1) * 128],
                        identity=ident[0:64, 0:64],
                    )
                    t = mpool.tile([128, 64], DT, tag=f"hT{kc}")
                    nc.vector.tensor_copy(out=t[:], in_=pt[:, 0:64])
                    hT.append(t)

                # Pstack = [h @ W_A ; h @ W_B]  (lhsT for the L1 one-hot matmul).
                # f32r matmuls must write psum at partition 0, so the W_B half
                # bounces through SBUF and a partition-shifting DMA.
                Pstack = mpool.tile([128, 512], DT, tag="Pstack")
                ptmp = mpool.tile([64, 512], DT, tag="ptmp")
                for half, W in ((0, W_A), (1, W_B)):
                    pp = pbig.tile([128, 512], F32, tag="pbig")
                    for kc in range(2):
                        nc.tensor.matmul(
                            out=pp[0:64, :], lhsT=hT[kc][:], rhs=W[kc][:],
                            start=(kc == 0), stop=(kc == 1),
                        )
                    if half == 0:
                        nc.vector.tensor_copy(out=Pstack[0:64, :], in_=pp[0:64, :])
                    else:
                        nc.vector.tensor_copy(out=ptmp[:], in_=pp[0:64, :])
                        nc.sync.dma_start(out=Pstack[64:128, :], in_=ptmp[:])

                aggc = mpool.tile([64, 260], F32, tag="aggc")
                nc.gpsimd.memset(aggc[:], 0.0)

                for g in range(NEC // GRP):
                    stash = []
                    for ec in range(g * GRP, (g + 1) * GRP):
                        e0 = ec * TE
                        ne = min(TE, E - e0)
                        last = ne < TE

                        # ---- indices ----
                        iijj_r = epool.tile([2, TE], I32, tag="iijj_r")
                        if last:
                            nc.gpsimd.memset(iijj_r[:, ne:TE], -1)
                        nc.sync.dma_start(
                            out=iijj_r[0:1, 0:ne], in_=ei_in[m, e0:e0 + ne, 0][None, :]
                        )
                        nc.sync.dma_start(
                            out=iijj_r[1:2, 0:ne], in_=ei_in[m, e0:e0 + ne, 1][None, :]
                        )
                        iijj_f = epool.tile([2, TE], DT, tag="iijj_f")
                        nc.vector.tensor_copy(out=iijj_f[:], in_=iijj_r[:])

                        iic_i = epool.tile([128, 4], I32, tag="iic_i")
                        if last:
                            nc.gpsimd.memset(iic_i[:], -1)
                            nc.sync.dma_start(
                                out=iic_i[:, 0:3],
                                in_=ei_in[m, e0:e0 + 384, 0].rearrange(
                                    "(c p) -> p c", p=128
                                ),
                            )
                            nc.sync.dma_start(
                                out=iic_i[0:64, 3:4],
                                in_=ei_in[m, e0 + 384:e0 + 448, 0][:, None],
                            )
                        else:
                            nc.sync.dma_start(
                                out=iic_i[:],
                                in_=ei_in[m, e0:e0 + TE, 0].rearrange(
                                    "(c p) -> p c", p=128
                                ),
                            )
                        iic = epool.tile([128, 4], F32, tag="iic")
                        nc.vector.tensor_copy(out=iic[:], in_=iic_i[:])

                        # ---- one-hot (gather layout): rows 0-63 ii, 64-127 jj
                        pb = pbig.tile([128, TE], F32, tag="pbig")
                        nc.tensor.matmul(
                            out=pb[:], lhsT=sel2[:], rhs=iijj_f[:],
                            start=True, stop=True,
                        )
                        OT = epool.tile([128, TE], DT, tag="OT")
                        nc.vector.tensor_scalar(
                            out=OT[:], in0=pb[:], scalar1=iota2[:, 0:1],
                            scalar2=None, op0=OP.is_equal,
                        )

                        # scatter one-hot (edge-partition layout)
                        O_i = ppool.tile([128, 256], DT, tag="O_i")
                        for c in range(4):
                            nc.vector.tensor_scalar(
                                out=O_i[:, c * 64:(c + 1) * 64], in0=iotam[:],
                                scalar1=iic[:, c:c + 1], scalar2=None,
                                op0=OP.is_equal,
                            )

                        # ---- geometry ----
                        mask3 = epool.tile([3, TE], F32, tag="mask3")
                        if last:
                            nc.gpsimd.memset(mask3[:], 0.0)
                        for r in range(3):
                            nc.sync.dma_start(
                                out=mask3[r:r + 1, 0:ne],
                                in_=em_in[m, e0:e0 + ne][None, :],
                            )

                        geo = pgeo.tile([3, TE], F32, tag="geo")
                        nc.tensor.matmul(
                            out=geo[:], lhsT=xs[:], rhs=OT[:],
                            start=True, stop=True,
                        )
                        diff_m = ppool.tile([3, TE], F32, tag="diff_m")
                        nc.vector.tensor_mul(out=diff_m[:], in0=geo[:], in1=mask3[:])

                        sqa = epool.tile([7, TE], DT, tag="sqa")
                        nc.vector.tensor_mul(
                            out=sqa[0:3, :], in0=diff_m[:], in1=diff_m[:]
                        )
                        nc.gpsimd.dma_start(
                            out=sqa[3:7, 0:ne],
                            in_=a_in[m, e0:e0 + ne, :].rearrange("e k -> k e"),
                        )

                        # d^2 into a fresh geo bank, then stash in SBUF for the
                        # batched sqrt phase
                        geo2 = pgeo.tile([3, TE], F32, tag="geo")
                        nc.tensor.matmul(
                            out=geo2[:], lhsT=ones3[:], rhs=sqa[0:3, :],
                            start=True, stop=True,
                        )
                        d2_sb = ppool.tile([3, TE], F32, tag="d2_sb")
                        nc.vector.tensor_copy(out=d2_sb[:], in_=geo2[:])

                        # ---- layer 1 ----
                        m1 = []
                        for mc in range(4):
                            ms = slice(mc * 128, (mc + 1) * 128)
                            pl1 = pbig.tile([128, TE], F32, tag="pbig")
                            nc.tensor.matmul(
                                out=pl1[:], lhsT=Pstack[:, ms], rhs=OT[:],
                                start=True, stop=False,
                            )
                            nc.tensor.matmul(
                                out=pl1[:], lhsT=W_sqa[:, ms], rhs=sqa[:],
                                start=False, stop=True,
                            )
                            t = epool.tile([128, TE], DT, tag=f"m1_{mc}")
                            bcol = (BE1 if mc < 2 else BX1) + (mc % 2)
                            nc.scalar.activation(
                                out=t[:], in_=pl1[:], func=AF.Silu,
                                bias=BIAS[:, bcol:bcol + 1],
                            )
                            m1.append(t)

                        # ---- layer 2 ----
                        m2 = []
                        for path, (W, boff, srcs) in enumerate(
                            ((We2, BE2, m1[0:2]), (Wx2, BX2, m1[2:4]))
                        ):
                            for mc in range(2):
                                ms = slice(mc * 128, (mc + 1) * 128)
                                pl2 = pbig.tile([128, TE], F32, tag="pbig")
                                for kc in range(2):
                                    nc.tensor.matmul(
                                        out=pl2[:], lhsT=W[kc][:, ms],
                                        rhs=srcs[kc][:],
                                        start=(kc == 0), stop=(kc == 1),
                                    )
                                t = epool.tile([128, TE], DT, tag=f"m2_{path}_{mc}")
                                nc.scalar.activation(
                                    out=t[:], in_=pl2[:], func=AF.Silu,
                                    bias=BIAS[:, boff + mc:boff + mc + 1],
                                )
                                m2.append(t)
                        m2e, m2x = m2[0:2], m2[2:4]

                        # ---- gates (sigmoid via tanh: sigmoid(z) =
                        # 0.5*tanh(z/2) + 0.5 keeps the silu/tanh ACT table
                        # resident; BIAS col BA holds ba/2) ----
                        pg = pbig.tile([128, TE], F32, tag="pbig")
                        for kc in range(2):
                            nc.tensor.matmul(
                                out=pg[:], lhsT=Wa_r[kc][:], rhs=m2e[kc][:],
                                start=(kc == 0), stop=(kc == 1),
                            )
                        gate = epool.tile([128, TE], F32, tag="gate")
                        nc.scalar.activation(
                            out=gate[:], in_=pg[:], func=AF.Tanh,
                            bias=BIAS[:, BA:BA + 1], scale=0.5,
                        )
                        nc.vector.tensor_scalar(
                            out=gate[:], in0=gate[:], scalar1=0.5, scalar2=0.5,
                            op0=OP.mult, op1=OP.add,
                        )
                        em = []
                        for kc in range(2):
                            t = ppool.tile([128, TE], DT, tag=f"em_{kc}")
                            nc.vector.tensor_mul(out=t[:], in0=m2e[kc][:], in1=gate[:])
                            em.append(t)

                        px = pbig.tile([128, TE], F32, tag="pbig")
                        for kc in range(2):
                            nc.tensor.matmul(
                                out=px[:], lhsT=Wx3_r[kc][:], rhs=m2x[kc][:],
                                start=(kc == 0), stop=(kc == 1),
                            )
                        xw3 = ppool.tile([3, TE], F32, tag="xw3")
                        nc.scalar.activation(out=xw3[:], in_=px[0:3, :], func=AF.Tanh)

                        stash.append((em, O_i, diff_m, d2_sb, xw3))

                    # ---- phase B: batched sqrt + xm + scatter for the group ----
                    for gi, (em, O_i, diff_m, d2_sb, xw3) in enumerate(stash):
                        r15 = epool.tile([3, TE], F32, tag="r15")
                        nc.scalar.sqrt(out=r15[:], in_=d2_sb[:])
                        nc.vector.tensor_scalar(
                            out=r15[:], in0=r15[:], scalar1=1.0, scalar2=None,
                            op0=OP.add,
                        )
                        nc.vector.reciprocal(out=r15[:], in_=r15[:])

                        xm = epool.tile([4, TE], F32, tag="xm")
                        nc.gpsimd.memset(xm[:], 0.0)
                        nc.vector.tensor_mul(out=xm[0:3, :], in0=xw3[:], in1=diff_m[:])
                        nc.vector.tensor_mul(out=xm[0:3, :], in0=xm[0:3, :], in1=r15[:])
                        nc.vector.tensor_scalar(
                            out=xm[0:3, :], in0=xm[0:3, :], scalar1=SCALE,
                            scalar2=None, op0=OP.mult,
                        )

                        # merged em+xm scatter: one [64, 260] psum group per
                        # edge tile, drained into the SBUF accumulator
                        psc = pagg.tile([64, 260], F32, tag="pagg")
                        for c in range(4):
                            cs = slice(c * 128, (c + 1) * 128)
                            emx = epool.tile([128, 260], DT, tag="emx")
                            for kc in range(2):
                                pt = ppt.tile([128, 128], DT, tag="pt")
                                nc.tensor.transpose(
                                    out=pt[:], in_=em[kc][:, cs], identity=ident_r[:],
                                )
                                nc.scalar.copy(
                                    out=emx[:, kc * 128:(kc + 1) * 128], in_=pt[:]
                                )
                            ptx = ppt.tile([128, 128], F32, tag="pt")
                            nc.tensor.transpose(
                                out=ptx[:, 0:4], in_=xm[:, cs],
                                identity=ident[0:4, 0:4],
                            )
                            nc.scalar.copy(out=emx[:, 256:260], in_=ptx[:, 0:4])
                            nc.tensor.matmul(
                                out=psc[:], lhsT=O_i[:, c * 64:(c + 1) * 64],
                                rhs=emx[:], start=(c == 0), stop=(c == 3),
                                skip_group_check=True,
                            )
                        nc.vector.tensor_add(out=aggc[:], in0=aggc[:], in1=psc[:])

                # ---- node MLP + outputs ----
                em_aggT = []
                for kc in range(2):
                    pt = ppt.tile([128, 128], F32, tag="pt")
                    nc.tensor.transpose(
                        out=pt[:, 0:64],
                        in_=aggc[:, kc * 128:(kc + 1) * 128],
                        identity=ident[0:64, 0:64],
                    )
                    t = mpool.tile([128, 64], DT, tag=f"em_aggT{kc}")
                    nc.vector.tensor_copy(out=t[:], in_=pt[:, 0:64])
                    em_aggT.append(t)

                t1s = []
                for mc in range(2):
                    ms = slice(mc * 128, (mc + 1) * 128)
                    pz = ppt.tile([128, 128], F32, tag="pt")
                    for kc in range(2):
                        nc.tensor.matmul(
                            out=pz[:, 0:64], lhsT=Wh1a[kc][:, ms], rhs=hT[kc][:],
                            start=(kc == 0), stop=False,
                        )
                    for kc in range(2):
                        nc.tensor.matmul(
                            out=pz[:, 0:64], lhsT=Wh1b[kc][:, ms],
                            rhs=em_aggT[kc][:],
                            start=False, stop=(kc == 1),
                        )
                    t = mpool.tile([128, 64], DT, tag=f"t1s{mc}")
                    nc.scalar.activation(
                        out=t[:], in_=pz[:, 0:64], func=AF.Silu,
                        bias=BIAS[:, BH1 + mc:BH1 + mc + 1],
                    )
                    t1s.append(t)

                t2 = mpool.tile([64, H], F32, tag="t2")
                for mc in range(2):
                    ms = slice(mc * 128, (mc + 1) * 128)
                    pz = ppt.tile([128, 128], F32, tag="pt")
                    for kc in range(2):
                        nc.tensor.matmul(
                            out=pz[:, 0:64], lhsT=Wh2[kc][:, ms], rhs=t1s[kc][:],
                            start=(kc == 0), stop=(kc == 1),
                        )
                    t2T = mpool.tile([128, 64], F32, tag=f"t2T{mc}")
                    nc.vector.tensor_scalar(
                        out=t2T[:], in0=pz[:, 0:64],
                        scalar1=BIAS[:, BH2 + mc:BH2 + mc + 1],
                        scalar2=None, op0=OP.add,
                    )
                    pb2 = ppt.tile([128, 128], F32, tag="pt")
                    nc.tensor.transpose(
                        out=pb2[0:64, :], in_=t2T[:], identity=ident[:],
                    )
                    nc.vector.tensor_copy(out=t2[:, ms], in_=pb2[0:64, :])

                nm = mpool.tile([64, 1], F32, tag="nm")
                nc.sync.dma_start(out=nm[:], in_=nm_in[m, :][:, None])

                ho = mpool.tile([64, H], F32, tag="ho")
                nc.vector.tensor_add(out=ho[:], in0=h_sb[:], in1=t2[:])
                nc.vector.tensor_scalar(
                    out=ho[:], in0=ho[:], scalar1=nm[:, 0:1], scalar2=None,
                    op0=OP.mult,
                )
                nc.sync.dma_start(out=h_out[m], in_=ho[:])

                xo = mpool.tile([64, 3], F32, tag="xo")
                nc.vector.tensor_add(out=xo[:], in0=xs[0:64, :], in1=aggc[:, 256:259])
                nc.vector.tensor_scalar(
                    out=xo[:], in0=xo[:], scalar1=nm[:, 0:1], scalar2=None,
                    op0=OP.mult,
                )
                nc.sync.dma_start(out=x_out[m], in_=xo[:])

    return nc


_NC_CACHE = None


def _get_nc():
    global _NC_CACHE
    if _NC_CACHE is None:
        _NC_CACHE = build_nc()
    return _NC_CACHE


def make_in_maps(x, h, a, edge_indices, node_mask, edge_mask,
                 We1, be1, We2, be2, Wa, ba,
                 Wh1, bh1, Wh2, bh2, Wx1, bx1, Wx2, bx2, Wx3):
    f32 = np.float32
    W_A = np.concatenate([We1[0:256], Wx1[0:256]], axis=1).astype(f32)
    W_B = np.concatenate([We1[256:512], Wx1[256:512]], axis=1).astype(f32)
    W_sqa = np.zeros((7, 512), f32)
    W_sqa[0:3, :] = np.concatenate([We1[512:513], Wx1[512:513]], axis=1)
    W_sqa[3:7, :] = np.concatenate([We1[513:517], Wx1[513:517]], axis=1)
    Wa_rep = np.repeat(Wa.astype(f32), 128, axis=1)
    Wx3_rep = np.repeat(Wx3.astype(f32), 128, axis=1)
    BIAS = np.zeros((128, 13), f32)
    for col, vec in ((0, be1), (2, bx1), (4, be2), (6, bx2), (8, bh1), (10, bh2)):
        BIAS[:, col] = vec[0:128]
        BIAS[:, col + 1] = vec[128:256]
    BIAS[:, 12] = 0.5 * ba[0]

    ONES = np.ones((3, 128), f32)
    SEL2 = np.zeros((2, 128), f32)
    SEL2[0, 0:64] = 1.0   # broadcast row 0 (ii) to partitions 0-63
    SEL2[1, 64:128] = 1.0  # broadcast row 1 (jj) to partitions 64-127
    shared = dict(
        W_A=W_A, W_B=W_B, W_sqa=W_sqa,
        We2=We2.astype(f32), Wx2=Wx2.astype(f32),
        Wh1a=Wh1[0:256].astype(f32), Wh1b=Wh1[256:512].astype(f32),
        Wh2=Wh2.astype(f32), Wa_rep=Wa_rep, Wx3_rep=Wx3_rep, BIAS=BIAS,
        ONES=ONES, SEL2=SEL2,
    )
    in_maps = []
    for c in range(NCORES):
        s = slice(c * BM, (c + 1) * BM)
        in_maps.append(dict(
            x_in=np.ascontiguousarray(x[s], f32),
            h_in=np.ascontiguousarray(h[s], f32),
            a_in=np.ascontiguousarray(a[s], f32),
            ei_in=np.ascontiguousarray(edge_indices[s], np.int32),
            nm_in=np.ascontiguousarray(node_mask[s, :, 0], f32),
            em_in=np.ascontiguousarray(edge_mask[s, :, 0], f32),
            **shared,
        ))
    return in_maps


def kernel(**inputs):
    inputs = {k: np.asarray(v) for k, v in inputs.items()}
    nc = _get_nc()
    in_maps = make_in_maps(**inputs)
    res = run_bass_kernel_spmd(nc, in_maps, list(range(NCORES)))
    x_out = np.concatenate([res.results[c]["x_out"] for c in range(NCORES)], axis=0)
    h_out = np.concatenate([res.results[c]["h_out"] for c in range(NCORES)], axis=0)
    return x_out.astype(np.float32), h_out.astype(np.float32)
